# revision 1
# baseline (speedup 1.0000x reference)
"""Trainium2 Bass kernel for CentroidsFlowAD (retrieval_knn, K=1).

Math: for each embedding row e (B*N rows of dim D=1024) and centroid bank
C [M=2048, D], the reference computes min_m sqrt(max(||e||^2 + ||c_m||^2
- 2 e.c_m, 0)). With K_NEIGHBORS=1 the softmin weighting is exactly 1, so
the output is just the distance to the nearest centroid, reshaped to
[B, 1, 56, 56].

Strategy (data-parallel over batch across 8 cores, centroids replicated):
  - host: split embeds by batch (4 samples -> 12544 rows per core),
    cast to bf16 and transpose to [D, R] so the contraction dim lands on
    SBUF partitions; precompute ||e||^2 (fp32) and ||c||^2/2 host-side.
  - device: cross = E^T tiles (stationary, [128d x 128r]) x C^T (moving,
    [128d x 512c]) accumulated over 8 k-chunks into PSUM [128r, 2048c];
    one fused DVE tensor_tensor_reduce computes max_m(cross - csq/2)
    per row; epilogue computes sqrt(max(feat - 2*hmax, eps)) with a
    Newton refinement of the ACT LUT sqrt.
  - host: gather per-core [128, NT] outputs, unpermute, reshape.

bf16 matmul with fp32 PSUM accumulation gives ~1.5e-4 max rel err vs the
fp32 reference (verified empirically); PE roofline ~670us/core.
"""

import numpy as np
import ml_dtypes

import concourse.bass as bass
import concourse.mybir as mybir
import concourse.tile as tile
from concourse import bacc
from concourse.bass_utils import run_bass_kernel_spmd

# Problem constants (hardcoded per harness contract)
B, N, D, M = 32, 3136, 1024, 2048
N_CORES = 8
B_PER_CORE = B // N_CORES            # 4
R = B_PER_CORE * N                   # 12544 rows per core
NT = R // 128                        # 98 row tiles per core
KC = D // 128                        # 8 contraction chunks
NC_CHUNKS = M // 512                 # 4 PSUM chunks of 512 centroids
FP_H = 56

BF16 = mybir.dt.bfloat16
F32 = mybir.dt.float32
NP_BF16 = ml_dtypes.bfloat16


def build_program(n_row_tiles=NT, block_tiles=14, n_iters=1, n_devices=N_CORES,
                  enable_asserts=False):
    """Build + compile the SPMD bass program.

    n_row_tiles: row tiles (128 rows each) processed per core.
    block_tiles: row tiles per DMA block (must divide n_row_tiles).
    n_iters: repeat whole compute (for loop-delta timing). >1 wraps in For_i.
    """
    assert n_row_tiles % block_tiles == 0
    n_blocks = n_row_tiles // block_tiles
    rows = n_row_tiles * 128
    blk = block_tiles * 128

    nc = bacc.Bacc("TRN2", target_bir_lowering=False, debug=False,
                   num_devices=n_devices, enable_asserts=enable_asserts)

    et = nc.dram_tensor("et", [D, rows], BF16, kind="ExternalInput").ap()
    ct = nc.dram_tensor("ct", [D, M], BF16, kind="ExternalInput").ap()
    csqh = nc.dram_tensor("csqh", [128, M], F32, kind="ExternalInput").ap()
    feat = nc.dram_tensor("feat", [128, n_row_tiles], F32,
                          kind="ExternalInput").ap()
    out = nc.dram_tensor("out", [128, n_row_tiles], F32,
                         kind="ExternalOutput").ap()

    with tile.TileContext(nc) as tc:
        with (
            tc.tile_pool(name="const", bufs=1) as const_pool,
            tc.tile_pool(name="etp", bufs=2) as et_pool,
            tc.tile_pool(name="psum", bufs=2, space="PSUM") as psum_pool,
            tc.tile_pool(name="junk", bufs=2) as junk_pool,
            tc.tile_pool(name="epi", bufs=1) as epi_pool,
        ):
            ct_sb = const_pool.tile([128, KC, M], BF16)
            csqh_sb = const_pool.tile([128, M], F32)
            feat_sb = const_pool.tile([128, n_row_tiles], F32)
            hmax_sb = const_pool.tile([128, n_row_tiles], F32)
            for k in range(KC):
                nc.sync.dma_start(ct_sb[:, k, :], ct[k * 128:(k + 1) * 128, :])
            nc.sync.dma_start(csqh_sb[:], csqh[:, :])
            nc.sync.dma_start(feat_sb[:], feat[:, :])

            def body(_it=None):
                for b in range(n_blocks):
                    et_sb = et_pool.tile([128, KC, blk], BF16)
                    for k in range(KC):
                        nc.sync.dma_start(
                            et_sb[:, k, :],
                            et[k * 128:(k + 1) * 128, b * blk:(b + 1) * blk])
                    for j in range(block_tiles):
                        t = b * block_tiles + j
                        ps = psum_pool.tile([128, M], F32)
                        for k in range(KC):
                            lhsT = et_sb[:, k, j * 128:(j + 1) * 128]
                            for n in range(NC_CHUNKS):
                                nc.tensor.matmul(
                                    ps[:, n * 512:(n + 1) * 512],
                                    lhsT,
                                    ct_sb[:, k, n * 512:(n + 1) * 512],
                                    start=(k == 0), stop=(k == KC - 1))
                        # (custom-ISA tensor_tensor_reduce is unsupported by
                        # this compile path; use two standard DVE ops)
                        h_sb = junk_pool.tile([128, M], F32)
                        nc.vector.tensor_sub(h_sb[:], ps[:], csqh_sb[:])
                        nc.vector.tensor_reduce(
                            hmax_sb[:, t:t + 1], h_sb[:],
                            mybir.AxisListType.X, mybir.AluOpType.max)

                # epilogue: dist = sqrt(max(feat - 2*hmax, eps)), Newton-refined
                d2 = epi_pool.tile([128, n_row_tiles], F32)
                nc.vector.scalar_tensor_tensor(
                    out=d2[:], in0=hmax_sb[:], scalar=-2.0, in1=feat_sb[:],
                    op0=mybir.AluOpType.mult, op1=mybir.AluOpType.add)
                d2c = epi_pool.tile([128, n_row_tiles], F32)
                nc.vector.tensor_scalar_max(d2c[:], d2[:], 1.0e-12)
                s0 = epi_pool.tile([128, n_row_tiles], F32)
                nc.scalar.activation(s0[:], d2c[:],
                                     mybir.ActivationFunctionType.Sqrt)
                rcp = epi_pool.tile([128, n_row_tiles], F32)
                nc.vector.reciprocal(rcp[:], s0[:])
                q = epi_pool.tile([128, n_row_tiles], F32)
                nc.vector.tensor_mul(q[:], d2c[:], rcp[:])
                sq = epi_pool.tile([128, n_row_tiles], F32)
                nc.vector.tensor_add(sq[:], s0[:], q[:])
                res = epi_pool.tile([128, n_row_tiles], F32)
                nc.vector.tensor_scalar_mul(res[:], sq[:], 0.5)
                nc.sync.dma_start(out[:, :], res[:])

            # python-unrolled repetitions (For_i's back-edge machinery has
            # crashed the exec unit on this terminal; unrolled is safe)
            for _ in range(n_iters):
                body()

    nc.compile()
    return nc


_NC_CACHE = {}


def _get_program(key=(NT, 14, 1, N_CORES)):
    if key not in _NC_CACHE:
        _NC_CACHE[key] = build_program(*key)
    return _NC_CACHE[key]


def prep_inputs(embeds, centroids):
    """Host-side shard + layout prep. Returns per-core input maps."""
    embeds = np.asarray(embeds)
    centroids = np.asarray(centroids)
    ct_np = np.ascontiguousarray(centroids.astype(NP_BF16).T)       # [D, M]
    csq = np.einsum("md,md->m", centroids.astype(np.float64),
                    centroids.astype(np.float64))
    csqh_rep = np.ascontiguousarray(
        np.broadcast_to((csq * 0.5).astype(np.float32)[None, :], (128, M)))
    in_maps = []
    for c in range(N_CORES):
        e = embeds[c * B_PER_CORE:(c + 1) * B_PER_CORE].reshape(R, D)
        et_np = np.ascontiguousarray(e.astype(NP_BF16).T)           # [D, R]
        f = np.einsum("rd,rd->r", e.astype(np.float64),
                      e.astype(np.float64)).astype(np.float32)
        feat_np = np.ascontiguousarray(f.reshape(NT, 128).T)        # [128, NT]
        in_maps.append({"et": et_np, "ct": ct_np, "csqh": csqh_rep,
                        "feat": feat_np})
    return in_maps


def gather_output(results):
    """results: list of 8 dicts with 'out' [128, NT] -> [B, 1, 56, 56]."""
    per_core = [np.asarray(r["out"]).T.reshape(R) for r in results]
    sim = np.concatenate(per_core).reshape(B, N)
    return sim.reshape(B, FP_H, FP_H)[:, None, :, :].astype(np.float32)


def kernel(embeds, centroids):
    nc = _get_program()
    in_maps = prep_inputs(embeds, centroids)
    res = run_bass_kernel_spmd(nc, in_maps, list(range(N_CORES)))
    return gather_output(res.results)


class CachedRunner:
    """Low-overhead repeat runner: jit once, keep inputs resident on device.

    Mirrors bass2jax.run_bass_via_pjrt's multi-core path but caches the
    jitted callable and the device-side input shards so repeated calls pay
    only dispatch + execution (for timing measurements).
    """

    def __init__(self, nc, in_maps):
        import jax
        import concourse.mybir as _mybir
        from jax.sharding import Mesh, PartitionSpec, NamedSharding
        from jax.experimental.shard_map import shard_map
        from concourse import bass2jax

        bass2jax.install_neuronx_cc_hook()
        n_cores = len(in_maps)
        partition_name = (nc.partition_id_tensor.name
                          if nc.partition_id_tensor else None)
        in_names, out_names, out_avals = [], [], []
        for alloc in nc.m.functions[0].allocations:
            if not isinstance(alloc, _mybir.MemoryLocationSet):
                continue
            name = alloc.memorylocations[0].name
            if alloc.kind == "ExternalInput":
                if name != partition_name:
                    in_names.append(name)
            elif alloc.kind == "ExternalOutput":
                shape = tuple(alloc.tensor_shape)
                dtype = _mybir.dt.np(alloc.dtype)
                out_names.append(name)
                out_avals.append(jax.core.ShapedArray(shape, dtype))
        n_params = len(in_names)
        all_in = in_names + out_names
        if partition_name is not None:
            all_in.append(partition_name)

        def _body(*args):
            operands = list(args)
            if partition_name is not None:
                operands.append(bass2jax.partition_id_tensor())
            outs = bass2jax._bass_exec_p.bind(
                *operands,
                out_avals=tuple(out_avals),
                in_names=tuple(all_in),
                out_names=tuple(out_names),
                lowering_input_output_aliases=(),
                sim_require_finite=True,
                sim_require_nnan=True,
                nc=nc,
            )
            return tuple(outs)

        devices = jax.devices()[:n_cores]
        mesh = Mesh(np.asarray(devices), ("core",))
        n_outs = len(out_names)
        donate = tuple(range(n_params, n_params + n_outs))
        self._fn = jax.jit(
            shard_map(_body, mesh=mesh,
                      in_specs=(PartitionSpec("core"),) * (n_params + n_outs),
                      out_specs=(PartitionSpec("core"),) * n_outs,
                      check_rep=False),
            donate_argnums=donate, keep_unused=True)
        sh = NamedSharding(mesh, PartitionSpec("core"))
        self._dev_in = [
            jax.device_put(
                np.concatenate([np.asarray(in_maps[c][nm])
                                for c in range(n_cores)], axis=0), sh)
            for nm in in_names]
        self._zero_shapes = [(n_cores * a.shape[0], *a.shape[1:])
                             for a in out_avals]
        self._zero_dtypes = [a.dtype for a in out_avals]
        self._out_names = out_names
        self._out_avals = out_avals
        self._n_cores = n_cores
        self._jax = jax

    def __call__(self):
        zeros = [np.zeros(s, d) for s, d in
                 zip(self._zero_shapes, self._zero_dtypes)]
        out = self._fn(*self._dev_in, *zeros)
        self._jax.block_until_ready(out)
        return out

    def results(self):
        out = self()
        return [
            {nm: np.asarray(out[i]).reshape(
                self._n_cores, *self._out_avals[i].shape)[c]
             for i, nm in enumerate(self._out_names)}
            for c in range(self._n_cores)]



# revision 6
# speedup vs baseline: 7.8696x; 7.8696x over previous
"""Trainium2 Bass kernel for CentroidsFlowAD (retrieval_knn, K=1).

Math: for each embedding row e (B*N rows of dim D=1024) and centroid bank
C [M=2048, D], the reference computes min_m sqrt(max(||e||^2 + ||c_m||^2
- 2 e.c_m, 0)). With K_NEIGHBORS=1 the softmin weighting is exactly 1, so
the output is just the distance to the nearest centroid, reshaped to
[B, 1, 56, 56].

Strategy (data-parallel over batch across 8 cores, centroids replicated):
  - host: split embeds by batch (4 samples -> 12544 rows per core),
    cast to fp8e4 (TRN E4M3) and transpose to [D, R] so the contraction
    dim lands on SBUF partitions; precompute ||e||^2 (fp32) and
    ||c||^2/2 host-side.
  - device: cross = E^T tiles (stationary, fp8 DoubleRow [128k x 2 x
    128r]) x C^T (moving, [128k x 2 x 512c]) accumulated over 4 K=256
    chunks into PSUM [128r, 2048c] fp32; one fused DVE
    tensor_tensor_reduce computes max_m(cross - csq/2) per row;
    epilogue computes sqrt(max(feat - 2*hmax, eps)) with a Newton
    refinement of the ACT LUT sqrt.
  - host: gather per-core [128, NT] outputs, unpermute, reshape.

fp8e4 DoubleRow runs the PE at 2x bf16 rate (2 MACs/cell/cycle); input
quantization noise gives ~4e-3 max rel err vs the fp32 reference, well
inside the 2e-2 gate.

Reduction pipeline (the PSUM scan is the second bottleneck after the PE):
the DVE reads PSUM fp32 at only 1 elem/cycle/partition @0.96 GHz, so a
naive sub+reduce costs ~4.3us/tile. Instead the ACT engine (1.2 GHz)
converts most of the PSUM tile to bf16 in SBUF, and the DVE does the
(h = cross - csq/2, max_m h) scan on bf16 at 2x/4x packing; a small
fp32 head chunk stays on the DVE to balance the two engines. csq/2 is
shifted by 512 (folded into feat) so bf16 values sit near zero and
rounding error stays ~1 ulp of ~128 (≲3e-4 final rel err).
(The fused custom-ISA tensor_tensor_reduce op compiles + simulates but
crashes the runtime on this exec path — verified, do not use.)
"""

import numpy as np
import ml_dtypes

import concourse.bass as bass
import concourse.mybir as mybir
import concourse.tile as tile
from concourse import bacc
from concourse.bass_utils import run_bass_kernel_spmd

# Problem constants (hardcoded per harness contract)
B, N, D, M = 32, 3136, 1024, 2048
N_CORES = 8
B_PER_CORE = B // N_CORES            # 4
R = B_PER_CORE * N                   # 12544 rows per core
NT = R // 128                        # 98 row tiles per core
KC = D // 128                        # 8 contraction chunks of 128
KC2 = KC // 2                        # 4 DoubleRow chunks of 256
NC_CHUNKS = M // 512                 # 4 PSUM chunks of 512 centroids
FP_H = 56

FP8 = mybir.dt.float8e4
F32 = mybir.dt.float32
BF16 = mybir.dt.bfloat16
NP_FP8 = ml_dtypes.float8_e4m3
NP_BF16 = ml_dtypes.bfloat16
DR = mybir.MatmulPerfMode.DoubleRow

CSQ_SHIFT = 512.0   # csq/2 is stored shifted by this; folded into feat
C0_DEFAULT = 256    # columns of the PSUM scan kept on the DVE in fp32


def build_program(n_row_tiles=NT, block_tiles=14, n_iters=1, n_devices=N_CORES,
                  enable_asserts=False, c0=C0_DEFAULT):
    """Build + compile the SPMD bass program.

    n_row_tiles: row tiles (128 rows each) processed per core.
    block_tiles: row tiles per DMA block (must divide n_row_tiles).
    n_iters: repeat whole compute (for loop-delta timing), python-unrolled.
    c0: leading PSUM columns handled by the DVE directly in fp32; the
        remaining 2048-c0 go through the ACT bf16-convert path.
    """
    assert n_row_tiles % block_tiles == 0
    n_blocks = n_row_tiles // block_tiles
    rows = n_row_tiles * 128
    blk = block_tiles * 128

    nc = bacc.Bacc("TRN2", target_bir_lowering=False, debug=False,
                   num_devices=n_devices, enable_asserts=enable_asserts)

    cf = M - c0  # columns through the ACT bf16 path

    et = nc.dram_tensor("et", [D, rows], FP8, kind="ExternalInput").ap()
    ct = nc.dram_tensor("ct", [D, M], FP8, kind="ExternalInput").ap()
    csqh = nc.dram_tensor("csqh", [128, M], F32, kind="ExternalInput").ap()
    csqhb = nc.dram_tensor("csqhb", [128, M], BF16, kind="ExternalInput").ap()
    feat = nc.dram_tensor("feat", [128, n_row_tiles], F32,
                          kind="ExternalInput").ap()
    out = nc.dram_tensor("out", [128, n_row_tiles], F32,
                         kind="ExternalOutput").ap()

    with tile.TileContext(nc) as tc:
        with (
            tc.tile_pool(name="const", bufs=1) as const_pool,
            tc.tile_pool(name="etp", bufs=2) as et_pool,
            tc.tile_pool(name="psum", bufs=2, space="PSUM") as psum_pool,
            tc.tile_pool(name="cb", bufs=3) as cb_pool,
            tc.tile_pool(name="hb", bufs=3) as hb_pool,
            tc.tile_pool(name="epi", bufs=1) as epi_pool,
        ):
            ct_sb = const_pool.tile([128, KC, M], FP8)
            csqh_sb = const_pool.tile([128, M], F32)
            csqhb_sb = const_pool.tile([128, M], BF16)
            feat_sb = const_pool.tile([128, n_row_tiles], F32)
            hmax1_sb = const_pool.tile([128, n_row_tiles], F32)
            hmax2_sb = const_pool.tile([128, n_row_tiles], F32)
            for k in range(KC):
                nc.sync.dma_start(ct_sb[:, k, :], ct[k * 128:(k + 1) * 128, :])
            nc.sync.dma_start(csqh_sb[:], csqh[:, :])
            nc.sync.dma_start(csqhb_sb[:], csqhb[:, :])
            nc.sync.dma_start(feat_sb[:], feat[:, :])

            def body(_it=None):
                for b in range(n_blocks):
                    et_sb = et_pool.tile([128, KC, blk], FP8)
                    for k in range(KC):
                        nc.sync.dma_start(
                            et_sb[:, k, :],
                            et[k * 128:(k + 1) * 128, b * blk:(b + 1) * blk])
                    for j in range(block_tiles):
                        t = b * block_tiles + j
                        ps = psum_pool.tile([128, M], F32)
                        for k2 in range(KC2):
                            lhsT = et_sb[:, 2 * k2:2 * k2 + 2,
                                         j * 128:(j + 1) * 128]
                            for n in range(NC_CHUNKS):
                                nc.tensor.matmul(
                                    ps[:, n * 512:(n + 1) * 512],
                                    lhsT,
                                    ct_sb[:, 2 * k2:2 * k2 + 2,
                                          n * 512:(n + 1) * 512],
                                    start=(k2 == 0), stop=(k2 == KC2 - 1),
                                    perf_mode=DR)
                        # h = cross - (csq/2 - 512); hmax = max_m h.
                        # fp32 head on DVE straight from PSUM:
                        if c0 > 0:
                            hb0 = hb_pool.tile([128, c0], BF16)
                            nc.vector.tensor_sub(hb0[:], ps[:, 0:c0],
                                                 csqh_sb[:, 0:c0])
                            nc.vector.tensor_reduce(
                                hmax1_sb[:, t:t + 1], hb0[:],
                                mybir.AxisListType.X, mybir.AluOpType.max)
                        # bf16 tail: ACT converts PSUM->bf16, DVE packs 2x/4x
                        if cf > 0:
                            cb = cb_pool.tile([128, cf], BF16)
                            nc.scalar.activation(
                                cb[:], ps[:, c0:M],
                                mybir.ActivationFunctionType.Copy)
                            hb1 = hb_pool.tile([128, cf], BF16)
                            nc.vector.tensor_sub(hb1[:], cb[:],
                                                 csqhb_sb[:, c0:M])
                            nc.vector.tensor_reduce(
                                hmax2_sb[:, t:t + 1], hb1[:],
                                mybir.AxisListType.X, mybir.AluOpType.max)

                # epilogue: dist = sqrt(max(feat' - 2*hmax, eps)), Newton-refined
                hmax = epi_pool.tile([128, n_row_tiles], F32)
                if c0 > 0 and cf > 0:
                    nc.vector.tensor_tensor(hmax[:], hmax1_sb[:], hmax2_sb[:],
                                            mybir.AluOpType.max)
                else:
                    src = hmax1_sb if c0 > 0 else hmax2_sb
                    nc.vector.tensor_scalar_mul(hmax[:], src[:], 1.0)
                d2 = epi_pool.tile([128, n_row_tiles], F32)
                nc.vector.scalar_tensor_tensor(
                    out=d2[:], in0=hmax[:], scalar=-2.0, in1=feat_sb[:],
                    op0=mybir.AluOpType.mult, op1=mybir.AluOpType.add)
                d2c = epi_pool.tile([128, n_row_tiles], F32)
                nc.vector.tensor_scalar_max(d2c[:], d2[:], 1.0e-12)
                s0 = epi_pool.tile([128, n_row_tiles], F32)
                nc.scalar.activation(s0[:], d2c[:],
                                     mybir.ActivationFunctionType.Sqrt)
                rcp = epi_pool.tile([128, n_row_tiles], F32)
                nc.vector.reciprocal(rcp[:], s0[:])
                q = epi_pool.tile([128, n_row_tiles], F32)
                nc.vector.tensor_mul(q[:], d2c[:], rcp[:])
                sq = epi_pool.tile([128, n_row_tiles], F32)
                nc.vector.tensor_add(sq[:], s0[:], q[:])
                res = epi_pool.tile([128, n_row_tiles], F32)
                nc.vector.tensor_scalar_mul(res[:], sq[:], 0.5)
                nc.sync.dma_start(out[:, :], res[:])

            # python-unrolled repetitions (For_i's back-edge machinery has
            # crashed the exec unit on this terminal; unrolled is safe)
            for _ in range(n_iters):
                body()

    nc.compile()
    return nc


_NC_CACHE = {}


def _get_program(key=(NT, 14, 1, N_CORES)):
    if key not in _NC_CACHE:
        _NC_CACHE[key] = build_program(*key)
    return _NC_CACHE[key]


def prep_inputs(embeds, centroids):
    """Host-side shard + layout prep. Returns per-core input maps."""
    embeds = np.asarray(embeds)
    centroids = np.asarray(centroids)
    ct_np = np.ascontiguousarray(centroids.astype(NP_FP8).T)        # [D, M]
    csq = np.einsum("md,md->m", centroids.astype(np.float64),
                    centroids.astype(np.float64))
    csqh_shift = (csq * 0.5 - CSQ_SHIFT).astype(np.float32)
    csqh_rep = np.ascontiguousarray(
        np.broadcast_to(csqh_shift[None, :], (128, M)))
    csqhb_rep = np.ascontiguousarray(
        np.broadcast_to(csqh_shift.astype(NP_BF16)[None, :], (128, M)))
    in_maps = []
    for c in range(N_CORES):
        e = embeds[c * B_PER_CORE:(c + 1) * B_PER_CORE].reshape(R, D)
        et_np = np.ascontiguousarray(e.astype(NP_FP8).T)            # [D, R]
        f = np.einsum("rd,rd->r", e.astype(np.float64),
                      e.astype(np.float64)).astype(np.float32)
        f += 2.0 * CSQ_SHIFT
        feat_np = np.ascontiguousarray(f.reshape(NT, 128).T)        # [128, NT]
        in_maps.append({"et": et_np, "ct": ct_np, "csqh": csqh_rep,
                        "csqhb": csqhb_rep, "feat": feat_np})
    return in_maps


def gather_output(results):
    """results: list of 8 dicts with 'out' [128, NT] -> [B, 1, 56, 56]."""
    per_core = [np.asarray(r["out"]).T.reshape(R) for r in results]
    sim = np.concatenate(per_core).reshape(B, N)
    return sim.reshape(B, FP_H, FP_H)[:, None, :, :].astype(np.float32)


def kernel(embeds, centroids):
    nc = _get_program()
    in_maps = prep_inputs(embeds, centroids)
    res = run_bass_kernel_spmd(nc, in_maps, list(range(N_CORES)))
    return gather_output(res.results)


class CachedRunner:
    """Low-overhead repeat runner: jit once, keep inputs resident on device.

    Mirrors bass2jax.run_bass_via_pjrt's multi-core path but caches the
    jitted callable and the device-side input shards so repeated calls pay
    only dispatch + execution (for timing measurements).
    """

    def __init__(self, nc, in_maps):
        import jax
        import concourse.mybir as _mybir
        from jax.sharding import Mesh, PartitionSpec, NamedSharding
        from jax.experimental.shard_map import shard_map
        from concourse import bass2jax

        bass2jax.install_neuronx_cc_hook()
        n_cores = len(in_maps)
        partition_name = (nc.partition_id_tensor.name
                          if nc.partition_id_tensor else None)
        in_names, out_names, out_avals = [], [], []
        for alloc in nc.m.functions[0].allocations:
            if not isinstance(alloc, _mybir.MemoryLocationSet):
                continue
            name = alloc.memorylocations[0].name
            if alloc.kind == "ExternalInput":
                if name != partition_name:
                    in_names.append(name)
            elif alloc.kind == "ExternalOutput":
                shape = tuple(alloc.tensor_shape)
                dtype = _mybir.dt.np(alloc.dtype)
                out_names.append(name)
                out_avals.append(jax.core.ShapedArray(shape, dtype))
        n_params = len(in_names)
        all_in = in_names + out_names
        if partition_name is not None:
            all_in.append(partition_name)

        def _body(*args):
            operands = list(args)
            if partition_name is not None:
                operands.append(bass2jax.partition_id_tensor())
            outs = bass2jax._bass_exec_p.bind(
                *operands,
                out_avals=tuple(out_avals),
                in_names=tuple(all_in),
                out_names=tuple(out_names),
                lowering_input_output_aliases=(),
                sim_require_finite=True,
                sim_require_nnan=True,
                nc=nc,
            )
            return tuple(outs)

        devices = jax.devices()[:n_cores]
        mesh = Mesh(np.asarray(devices), ("core",))
        n_outs = len(out_names)
        donate = tuple(range(n_params, n_params + n_outs))
        self._fn = jax.jit(
            shard_map(_body, mesh=mesh,
                      in_specs=(PartitionSpec("core"),) * (n_params + n_outs),
                      out_specs=(PartitionSpec("core"),) * n_outs,
                      check_rep=False),
            donate_argnums=donate, keep_unused=True)
        sh = NamedSharding(mesh, PartitionSpec("core"))
        self._dev_in = [
            jax.device_put(
                np.concatenate([np.asarray(in_maps[c][nm])
                                for c in range(n_cores)], axis=0), sh)
            for nm in in_names]
        self._zero_shapes = [(n_cores * a.shape[0], *a.shape[1:])
                             for a in out_avals]
        self._zero_dtypes = [a.dtype for a in out_avals]
        self._out_names = out_names
        self._out_avals = out_avals
        self._n_cores = n_cores
        self._jax = jax

    def __call__(self):
        zeros = [np.zeros(s, d) for s, d in
                 zip(self._zero_shapes, self._zero_dtypes)]
        out = self._fn(*self._dev_in, *zeros)
        self._jax.block_until_ready(out)
        return out

    def results(self):
        out = self()
        return [
            {nm: np.asarray(out[i]).reshape(
                self._n_cores, *self._out_avals[i].shape)[c]
             for i, nm in enumerate(self._out_names)}
            for c in range(self._n_cores)]


# revision 14
# speedup vs baseline: 12.5447x; 1.5941x over previous
"""Trainium2 Bass kernel for CentroidsFlowAD (retrieval_knn, K=1).

Math: for each embedding row e (B*N rows of dim D=1024) and centroid bank
C [M=2048, D], the reference computes min_m sqrt(max(||e||^2 + ||c_m||^2
- 2 e.c_m, 0)). With K_NEIGHBORS=1 the softmin weighting is exactly 1, so
the output is just the distance to the nearest centroid, reshaped to
[B, 1, 56, 56].

Strategy (data-parallel over batch across 8 cores, centroids replicated):
  - host: split embeds by batch (4 samples -> 12544 rows per core),
    cast to fp8e4 (TRN E4M3) and lay out as [128ki, k2, tile*2+ko, 128col]
    so every DoubleRow weight pair is contiguous in SBUF (pair stride
    128 B - large pair strides slow DR LDWEIGHTS 2.4-2.9x, measured);
    precompute ||e||^2 (fp32) and ||c||^2/2 host-side.
  - device: prefetch ALL inputs to SBUF (et is 98 KiB/partition at fp8,
    fits), then per 128-row tile: cross = E tile (stationary, fp8
    DoubleRow [128k x 2 x 128r]) x C^T (moving, [128k x 2 x 512c])
    accumulated over 4 K=256 chunks into PSUM [128r, 2048c] fp32;
    ACT/DVE-split reduction computes hmax = max_m(cross - csq/2);
    epilogue computes sqrt(max(feat - 2*hmax, eps)) with a Newton
    refinement of the ACT LUT sqrt.
  - host: gather per-core [128, NT] outputs, unpermute, reshape.

fp8e4 DoubleRow runs the PE at 2x bf16 rate (2 MACs/cell/cycle); input
quantization noise gives ~4e-3 max rel err vs the fp32 reference, well
inside the 2e-2 gate.

Reduction pipeline (the PSUM scan is the second bottleneck after the PE):
the DVE reads PSUM fp32 at only 1 elem/cycle/partition @0.96 GHz, so a
naive sub+reduce costs ~4.3us/tile. Instead the ACT engine (1.2 GHz)
converts most of the PSUM tile to bf16 in SBUF, and the DVE does the
(h = cross - csq/2, max_m h) scan on bf16 at 2x/4x packing; a small
fp32 head chunk stays on the DVE to balance the two engines. csq/2 is
shifted by 512 (folded into feat) so bf16 values sit near zero and
rounding error stays ~1 ulp of ~128 (≲3e-4 final rel err).
(The fused custom-ISA tensor_tensor_reduce op compiles + simulates but
crashes the runtime on this exec path — verified, do not use.)
"""

import numpy as np
import ml_dtypes

import concourse.bass as bass
import concourse.mybir as mybir
import concourse.tile as tile
from concourse import bacc
from concourse.bass_utils import run_bass_kernel_spmd

# Problem constants (hardcoded per harness contract)
B, N, D, M = 32, 3136, 1024, 2048
N_CORES = 8
B_PER_CORE = B // N_CORES            # 4
R = B_PER_CORE * N                   # 12544 rows per core
NT = R // 128                        # 98 row tiles per core
KC = D // 128                        # 8 contraction chunks of 128
KC2 = KC // 2                        # 4 DoubleRow chunks of 256
NC_CHUNKS = M // 512                 # 4 PSUM chunks of 512 centroids
FP_H = 56

FP8 = mybir.dt.float8e4
F32 = mybir.dt.float32
BF16 = mybir.dt.bfloat16
NP_FP8 = ml_dtypes.float8_e4m3
NP_BF16 = ml_dtypes.bfloat16
DR = mybir.MatmulPerfMode.DoubleRow

CSQ_SHIFT = 512.0   # csq/2 is stored shifted by this; folded into feat
C0_DEFAULT = 256    # columns of the PSUM scan kept on the DVE in fp32


def build_program(n_row_tiles=NT, block_tiles=14, n_iters=1, n_devices=N_CORES,
                  enable_asserts=False, c0=C0_DEFAULT, mode="full"):
    """Build + compile the SPMD bass program.

    n_row_tiles: row tiles (128 rows each) processed per core.
    block_tiles: row tiles per DMA block (must divide n_row_tiles).
    n_iters: repeat whole compute (for loop-delta timing), python-unrolled.
    c0: leading PSUM columns handled by the DVE directly in fp32; the
        remaining 2048-c0 go through the ACT bf16-convert path.
    mode: 'full' (real kernel) or engine-isolation experiments:
        'pe_only'  - matmuls + tiny DVE consume, no reduction
        'act_only' - matmuls + ACT convert + tiny DVE consume
        'dve_only' - matmuls + DVE fp32 sub + bf16 reduce (no ACT)
    """
    assert n_row_tiles % block_tiles == 0
    n_blocks = n_row_tiles // block_tiles
    rows = n_row_tiles * 128
    blk = block_tiles * 128

    nc = bacc.Bacc("TRN2", target_bir_lowering=False, debug=False,
                   num_devices=n_devices, enable_asserts=enable_asserts)

    cf = M - c0  # columns through the ACT bf16 path

    # et layout [ki=128, k2, t*2+ko, col]: the DoubleRow weight pair for
    # (k2, tile t) is CONTIGUOUS (pair stride 128 B). With the pair planes
    # far apart (e.g. [D, rows] layout, stride = rows bytes) the DR
    # LDWEIGHTS slows the stream from ~134 to ~323-385 ns/MM (measured).
    et = nc.dram_tensor("et", [128, KC2, n_row_tiles * 2, 128], FP8,
                        kind="ExternalInput").ap()
    ct = nc.dram_tensor("ct", [D, M], FP8, kind="ExternalInput").ap()
    csqh = nc.dram_tensor("csqh", [128, M], F32, kind="ExternalInput").ap()
    csqhb = nc.dram_tensor("csqhb", [128, M], BF16, kind="ExternalInput").ap()
    feat = nc.dram_tensor("feat", [128, n_row_tiles], F32,
                          kind="ExternalInput").ap()
    out = nc.dram_tensor("out", [128, n_row_tiles], F32,
                         kind="ExternalOutput").ap()

    with tile.TileContext(nc) as tc:
        with (
            tc.tile_pool(name="const", bufs=1) as const_pool,
            tc.tile_pool(name="psum", bufs=2, space="PSUM") as psum_pool,
            tc.tile_pool(name="cb", bufs=3) as cb_pool,
            tc.tile_pool(name="hb", bufs=3) as hb_pool,
            tc.tile_pool(name="epi", bufs=1) as epi_pool,
        ):
            ct_sb = const_pool.tile([128, KC, M], FP8)
            csqh_sb = const_pool.tile([128, M], F32)
            csqhb_sb = const_pool.tile([128, M], BF16)
            feat_sb = const_pool.tile([128, n_row_tiles], F32)
            hmax1_sb = const_pool.tile([128, n_row_tiles], F32)
            hmax2_sb = const_pool.tile([128, n_row_tiles], F32)
            # the full per-core et fits in SBUF (98 KiB/partition at fp8),
            # so prefetch everything up front: zero steady-state DMA means
            # zero SBUF-port interference with the PE weight/moving reads.
            et_sb = const_pool.tile([128, KC2, n_row_tiles * 2, 128], FP8)
            for k in range(KC):
                nc.sync.dma_start(ct_sb[:, k, :], ct[k * 128:(k + 1) * 128, :])
            nc.sync.dma_start(csqh_sb[:], csqh[:, :])
            nc.sync.dma_start(csqhb_sb[:], csqhb[:, :])
            nc.sync.dma_start(feat_sb[:], feat[:, :])
            # block-major DMA order so early tiles' weights land first
            for b in range(n_blocks):
                for k2 in range(KC2):
                    nc.sync.dma_start(
                        et_sb[:, k2, 2 * block_tiles * b:
                              2 * block_tiles * (b + 1), :],
                        et[:, k2, 2 * block_tiles * b:
                           2 * block_tiles * (b + 1), :])

            def body(_it=None):
                for b in range(n_blocks):
                    for j in range(block_tiles):
                        t = b * block_tiles + j
                        ps = psum_pool.tile([128, M], F32)
                        for k2 in range(KC2):
                            lhsT = et_sb[:, k2, 2 * t:2 * t + 2, :]
                            for n in range(NC_CHUNKS):
                                nc.tensor.matmul(
                                    ps[:, n * 512:(n + 1) * 512],
                                    lhsT,
                                    ct_sb[:, 2 * k2:2 * k2 + 2,
                                          n * 512:(n + 1) * 512],
                                    start=(k2 == 0), stop=(k2 == KC2 - 1),
                                    perf_mode=DR)
                        # h = cross - (csq/2 - 512); hmax = max_m h.
                        if mode == "pe_only":
                            nc.vector.tensor_scalar_mul(
                                hmax1_sb[:, t:t + 1], ps[:, 0:1], 1.0)
                            nc.vector.tensor_scalar_mul(
                                hmax2_sb[:, t:t + 1], ps[:, 0:1], 1.0)
                            continue
                        if mode == "act_only":
                            cb = cb_pool.tile([128, M], BF16)
                            nc.scalar.activation(
                                cb[:], ps[:],
                                mybir.ActivationFunctionType.Copy)
                            nc.vector.tensor_scalar_mul(
                                hmax1_sb[:, t:t + 1], cb[:, 0:1], 1.0)
                            nc.vector.tensor_scalar_mul(
                                hmax2_sb[:, t:t + 1], cb[:, 0:1], 1.0)
                            continue
                        if mode == "dve_only":
                            hb0 = hb_pool.tile([128, M], BF16)
                            nc.vector.tensor_sub(hb0[:], ps[:], csqh_sb[:])
                            nc.vector.tensor_reduce(
                                hmax1_sb[:, t:t + 1], hb0[:],
                                mybir.AxisListType.X, mybir.AluOpType.max)
                            nc.vector.tensor_scalar_mul(
                                hmax2_sb[:, t:t + 1], hmax1_sb[:, t:t + 1],
                                1.0)
                            continue
                        # fp32 head on DVE straight from PSUM:
                        if c0 > 0:
                            hb0 = hb_pool.tile([128, c0], BF16)
                            nc.vector.tensor_sub(hb0[:], ps[:, 0:c0],
                                                 csqh_sb[:, 0:c0])
                            nc.vector.tensor_reduce(
                                hmax1_sb[:, t:t + 1], hb0[:],
                                mybir.AxisListType.X, mybir.AluOpType.max)
                        # bf16 tail: ACT converts PSUM->bf16, DVE packs 2x/4x
                        if cf > 0:
                            cb = cb_pool.tile([128, cf], BF16)
                            nc.scalar.activation(
                                cb[:], ps[:, c0:M],
                                mybir.ActivationFunctionType.Copy)
                            hb1 = hb_pool.tile([128, cf], BF16)
                            nc.vector.tensor_sub(hb1[:], cb[:],
                                                 csqhb_sb[:, c0:M])
                            nc.vector.tensor_reduce(
                                hmax2_sb[:, t:t + 1], hb1[:],
                                mybir.AxisListType.X, mybir.AluOpType.max)

                # epilogue: dist = sqrt(max(feat' - 2*hmax, eps)), Newton-refined
                hmax = epi_pool.tile([128, n_row_tiles], F32)
                if c0 > 0 and cf > 0:
                    nc.vector.tensor_tensor(hmax[:], hmax1_sb[:], hmax2_sb[:],
                                            mybir.AluOpType.max)
                else:
                    src = hmax1_sb if c0 > 0 else hmax2_sb
                    nc.vector.tensor_scalar_mul(hmax[:], src[:], 1.0)
                d2 = epi_pool.tile([128, n_row_tiles], F32)
                nc.vector.scalar_tensor_tensor(
                    out=d2[:], in0=hmax[:], scalar=-2.0, in1=feat_sb[:],
                    op0=mybir.AluOpType.mult, op1=mybir.AluOpType.add)
                d2c = epi_pool.tile([128, n_row_tiles], F32)
                nc.vector.tensor_scalar_max(d2c[:], d2[:], 1.0e-12)
                s0 = epi_pool.tile([128, n_row_tiles], F32)
                nc.scalar.activation(s0[:], d2c[:],
                                     mybir.ActivationFunctionType.Sqrt)
                rcp = epi_pool.tile([128, n_row_tiles], F32)
                nc.vector.reciprocal(rcp[:], s0[:])
                q = epi_pool.tile([128, n_row_tiles], F32)
                nc.vector.tensor_mul(q[:], d2c[:], rcp[:])
                sq = epi_pool.tile([128, n_row_tiles], F32)
                nc.vector.tensor_add(sq[:], s0[:], q[:])
                res = epi_pool.tile([128, n_row_tiles], F32)
                nc.vector.tensor_scalar_mul(res[:], sq[:], 0.5)
                nc.sync.dma_start(out[:, :], res[:])

            # python-unrolled repetitions (For_i's back-edge machinery has
            # crashed the exec unit on this terminal; unrolled is safe)
            for _ in range(n_iters):
                body()

    nc.compile()
    return nc


_NC_CACHE = {}


def _get_program(key=(NT, 14, 1, N_CORES)):
    if key not in _NC_CACHE:
        _NC_CACHE[key] = build_program(*key)
    return _NC_CACHE[key]


def et_layout(e, n_tiles):
    """[rows, D] fp32 -> [128ki, KC2, n_tiles*2, 128col] fp8 with the
    DoubleRow pair planes (ko) adjacent per (k2, tile)."""
    x = e.astype(NP_FP8).reshape(n_tiles, 128, KC2, 2, 128)
    x = x.transpose(4, 2, 0, 3, 1)          # [ki, k2, t, ko, col]
    return np.ascontiguousarray(x.reshape(128, KC2, n_tiles * 2, 128))


def prep_inputs(embeds, centroids):
    """Host-side shard + layout prep. Returns per-core input maps."""
    embeds = np.asarray(embeds)
    centroids = np.asarray(centroids)
    ct_np = np.ascontiguousarray(centroids.astype(NP_FP8).T)        # [D, M]
    csq = np.einsum("md,md->m", centroids.astype(np.float64),
                    centroids.astype(np.float64))
    csqh_shift = (csq * 0.5 - CSQ_SHIFT).astype(np.float32)
    csqh_rep = np.ascontiguousarray(
        np.broadcast_to(csqh_shift[None, :], (128, M)))
    csqhb_rep = np.ascontiguousarray(
        np.broadcast_to(csqh_shift.astype(NP_BF16)[None, :], (128, M)))
    in_maps = []
    for c in range(N_CORES):
        e = embeds[c * B_PER_CORE:(c + 1) * B_PER_CORE].reshape(R, D)
        et_np = et_layout(e, NT)
        f = np.einsum("rd,rd->r", e.astype(np.float64),
                      e.astype(np.float64)).astype(np.float32)
        f += 2.0 * CSQ_SHIFT
        feat_np = np.ascontiguousarray(f.reshape(NT, 128).T)        # [128, NT]
        in_maps.append({"et": et_np, "ct": ct_np, "csqh": csqh_rep,
                        "csqhb": csqhb_rep, "feat": feat_np})
    return in_maps


def gather_output(results):
    """results: list of 8 dicts with 'out' [128, NT] -> [B, 1, 56, 56]."""
    per_core = [np.asarray(r["out"]).T.reshape(R) for r in results]
    sim = np.concatenate(per_core).reshape(B, N)
    return sim.reshape(B, FP_H, FP_H)[:, None, :, :].astype(np.float32)


def kernel(embeds, centroids):
    nc = _get_program()
    in_maps = prep_inputs(embeds, centroids)
    res = run_bass_kernel_spmd(nc, in_maps, list(range(N_CORES)))
    return gather_output(res.results)


class CachedRunner:
    """Low-overhead repeat runner: jit once, keep inputs resident on device.

    Mirrors bass2jax.run_bass_via_pjrt's multi-core path but caches the
    jitted callable and the device-side input shards so repeated calls pay
    only dispatch + execution (for timing measurements).
    """

    def __init__(self, nc, in_maps):
        import jax
        import concourse.mybir as _mybir
        from jax.sharding import Mesh, PartitionSpec, NamedSharding
        from jax.experimental.shard_map import shard_map
        from concourse import bass2jax

        bass2jax.install_neuronx_cc_hook()
        n_cores = len(in_maps)
        partition_name = (nc.partition_id_tensor.name
                          if nc.partition_id_tensor else None)
        in_names, out_names, out_avals = [], [], []
        for alloc in nc.m.functions[0].allocations:
            if not isinstance(alloc, _mybir.MemoryLocationSet):
                continue
            name = alloc.memorylocations[0].name
            if alloc.kind == "ExternalInput":
                if name != partition_name:
                    in_names.append(name)
            elif alloc.kind == "ExternalOutput":
                shape = tuple(alloc.tensor_shape)
                dtype = _mybir.dt.np(alloc.dtype)
                out_names.append(name)
                out_avals.append(jax.core.ShapedArray(shape, dtype))
        n_params = len(in_names)
        all_in = in_names + out_names
        if partition_name is not None:
            all_in.append(partition_name)

        def _body(*args):
            operands = list(args)
            if partition_name is not None:
                operands.append(bass2jax.partition_id_tensor())
            outs = bass2jax._bass_exec_p.bind(
                *operands,
                out_avals=tuple(out_avals),
                in_names=tuple(all_in),
                out_names=tuple(out_names),
                lowering_input_output_aliases=(),
                sim_require_finite=True,
                sim_require_nnan=True,
                nc=nc,
            )
            return tuple(outs)

        devices = jax.devices()[:n_cores]
        mesh = Mesh(np.asarray(devices), ("core",))
        n_outs = len(out_names)
        donate = tuple(range(n_params, n_params + n_outs))
        self._fn = jax.jit(
            shard_map(_body, mesh=mesh,
                      in_specs=(PartitionSpec("core"),) * (n_params + n_outs),
                      out_specs=(PartitionSpec("core"),) * n_outs,
                      check_rep=False),
            donate_argnums=donate, keep_unused=True)
        sh = NamedSharding(mesh, PartitionSpec("core"))
        self._dev_in = [
            jax.device_put(
                np.concatenate([np.asarray(in_maps[c][nm])
                                for c in range(n_cores)], axis=0), sh)
            for nm in in_names]
        self._zero_shapes = [(n_cores * a.shape[0], *a.shape[1:])
                             for a in out_avals]
        self._zero_dtypes = [a.dtype for a in out_avals]
        self._out_names = out_names
        self._out_avals = out_avals
        self._n_cores = n_cores
        self._jax = jax

    def __call__(self):
        zeros = [np.zeros(s, d) for s, d in
                 zip(self._zero_shapes, self._zero_dtypes)]
        out = self._fn(*self._dev_in, *zeros)
        self._jax.block_until_ready(out)
        return out

    def results(self):
        out = self()
        return [
            {nm: np.asarray(out[i]).reshape(
                self._n_cores, *self._out_avals[i].shape)[c]
             for i, nm in enumerate(self._out_names)}
            for c in range(self._n_cores)]
